# revision 2
# baseline (speedup 1.0000x reference)
"""CoAttention ImageDNS kernel v2 — transposed (weight-stationary) layout with
hybrid fp8 DoubleRow projections.

Math (same rank-1 insight as v1): softmax weights are query-independent:
  visual_att[b]  = softmax_r( wB . tanh(W_i1 @ img[b,r]) )
  textual_att[b] = softmax_j( wD . tanh(W_d2 @ dns[b,j]) )
so each output is a per-batch [H] attention-weighted average of rows.

v2 layout: projections run weight-stationary: lhsT = W^T chunk, rhs = x^T
chunk, so proj outputs land on PSUM *partitions* (o-dim) with rows on the
free dim.  tanh on Scalar, then the score reduction over the o-dim is a
small PE matmul with a broadcast wB column as stationary — output is the
score row replicated across all 128 partitions, so exp on Scalar directly
yields the broadcast attention-weight tile the stage-2 stts need (no
identity-matmul broadcast, no Vector copies; Vector only does stage-2).
Softmax sums come free from exp's accum_out (dns; img uses per-batch
Copy+accum).  Outputs are left unnormalized ([128, BLOC, HC] attc + per
batch sums); the host does the divide.

Speed: the first 4 of 8 contraction chunks run as fp8e4 DoubleRow pairs
(2 chunks per matmul, measured 2x MAC rate), the rest bf16; W is scaled
x32 before fp8 cast (tanh applies 1/32).  Rel err ~1.6-1.8e-2 (sim'd
exactly) vs the 2e-2 gate; stage-2 always reads bf16 x so the attended
average itself is full precision.  All DRAM inputs are host-prearranged
to the exact [128, free] SBUF layout so every load is a dense per-
partition run; loads are ordered critical-first on the sync/gpsimd
queues (Scalar's queue stays clear — its tanh stream gates the PE score
matmuls).
"""

import sys
import numpy as np
import ml_dtypes

_E4 = ml_dtypes.float8_e4m3
_BF16 = ml_dtypes.bfloat16

for _p in ("/opt/trn_rl_repo", "/root/.axon_site/_ro/trn_rl_repo"):
    if _p not in sys.path:
        sys.path.append(_p)

B, S, R, H = 32, 512, 196, 1024
NCORES = 8
BLOC = B // NCORES          # batches per core
HC = H // 128               # contraction chunks of 128
K8 = 4                      # leading chunks done in fp8 (DoubleRow pairs)
KB = HC - K8                # trailing chunks done in bf16
NI = BLOC * R               # img rows packed (784)
ND = BLOC * S               # dns rows packed (2048)
WSC = 32.0                  # fp8 weight pre-scale (tanh applies 1/WSC)

_CACHE = {}


def build_nc():
    from concourse import bacc, mybir
    from concourse import tile

    f32, f16, f8 = mybir.dt.float32, mybir.dt.bfloat16, mybir.dt.float8e4
    Act = mybir.ActivationFunctionType
    Alu = mybir.AluOpType
    PM = mybir.MatmulPerfMode

    nc = bacc.Bacc("TRN2", target_bir_lowering=False, debug=False)

    sides = ("img", "dns")
    NROW = {"img": NI, "dns": ND}
    # all inputs pre-arranged on host to [128 partitions, free] SBUF layout
    xt8_dram = {s: nc.dram_tensor(f"xt8_{s}", [128, K8, NROW[s]], f8,
                                  kind="ExternalInput") for s in sides}
    xt16_dram = {s: nc.dram_tensor(f"xt16_{s}", [128, HC, NROW[s]], f16,
                                   kind="ExternalInput") for s in sides}
    wt8_dram = {s: nc.dram_tensor(f"wt8_{s}", [128, K8 * H], f8,
                                  kind="ExternalInput") for s in sides}
    wt16_dram = {s: nc.dram_tensor(f"wt16_{s}", [128, KB * H], f16,
                                   kind="ExternalInput") for s in sides}
    wbp_dram = {s: nc.dram_tensor(f"wbp_{s}", [128, HC], f16,
                                  kind="ExternalInput") for s in sides}
    oatt = {s: nc.dram_tensor(f"oatt_{s}", [128, BLOC, HC], f32,
                              kind="ExternalOutput") for s in sides}
    NSUM = {"img": BLOC, "dns": BLOC + 4}   # dns b3 sum arrives as 4 partials
    osum = {s: nc.dram_tensor(f"osum_{s}", [128, NSUM[s]], f32,
                              kind="ExternalOutput") for s in sides}
    ident_dram = nc.dram_tensor("ident", [128, 128], f32, kind="ExternalInput")
    xn3_dram = nc.dram_tensor("xn3", [128, (S // 128) * H], f16,
                              kind="ExternalInput")
    out_b3 = nc.dram_tensor("out_b3", [1, H], f32, kind="ExternalOutput")

    with tile.TileContext(nc) as tc:
        with (
            tc.tile_pool(name="const", bufs=1) as cpool,
            tc.tile_pool(name="work", bufs=3) as wpool,
            tc.tile_pool(name="pp", bufs=4, space="PSUM") as ppool,
            tc.tile_pool(name="sc", bufs=3, space="PSUM") as scps,
            tc.tile_pool(name="wm", bufs=1, space="PSUM") as wmps,
        ):
            xt8 = {s: cpool.tile([128, K8 * NROW[s]], f8, name=f"xt8_{s}_sb")
                   for s in sides}
            xt16 = {s: cpool.tile([128, HC * NROW[s]], f16,
                                  name=f"xt16_{s}_sb") for s in sides}
            wt8 = {s: cpool.tile([128, K8 * H], f8, name=f"wt8_{s}_sb")
                   for s in sides}
            wt16 = {s: cpool.tile([128, KB * H], f16, name=f"wt16_{s}_sb")
                    for s in sides}
            wbp = {s: cpool.tile([128, HC], f16, name=f"wbp_{s}_sb")
                   for s in sides}
            xt8v = {s: xt8[s].rearrange("p (k m) -> p k m", k=K8)
                    for s in sides}
            xt16v = {s: xt16[s].rearrange("p (k m) -> p k m", k=HC)
                     for s in sides}
            wt8v = {s: wt8[s].rearrange("p (k m) -> p k m", k=K8)
                    for s in sides}
            wt16v = {s: wt16[s].rearrange("p (k m) -> p k m", k=KB)
                     for s in sides}
            aB_img = cpool.tile([128, NI], f16, name="aB_img")
            ident = cpool.tile([128, 128], f32, name="ident_sb")
            xn3 = cpool.tile([128, (S // 128) * H], f16, name="xn3_sb")
            xn3v = xn3.rearrange("p (g m) -> p g m", g=S // 128)
            aB3 = cpool.tile([128, S], f32, name="aB3_sb")
            acs3 = cpool.tile([128, S // 128], f16, name="acs3_sb")
            att3 = cpool.tile([128, H], f32, name="att3_sb")
            oatt_sb = {s: cpool.tile([128, BLOC * HC], f32,
                                     name=f"oatt_{s}_sb") for s in sides}
            osum_sb = {s: cpool.tile([128, NSUM[s]], f32, name=f"osum_{s}_sb")
                       for s in sides}

            # warm-up matmuls on memset data bridge the initial DMA wait so
            # the HAM clock gate reaches 8/8 before the first projection
            warm = cpool.tile([128, 256], f16, name="warm_sb")
            nc.vector.memset(warm[:, :], 1.0)
            wps = wmps.tile([128, 512], f32, name="warm_ps", tag="wm")
            for _ in range(24):
                nc.tensor.matmul(wps[:, 0:256], lhsT=warm[:, 0:128],
                                 rhs=warm[:, 0:256], start=True, stop=True)

            # ---- DMA loads: one global priority list, sliced to ~128-512KB
            # pieces, round-robin over the queues so each queue's FIFO
            # matches the global need order.  Scalar's queue only carries
            # pre-compute pieces (its DGE slices would delay the tanh
            # stream that gates the PE score matmuls).
            def ld_xt16(s, k0, k1, n0, n1):
                return lambda q: q.dma_start(
                    out=xt16v[s][:, k0:k1, n0:n1],
                    in_=xt16_dram[s][:, k0:k1, n0:n1])

            def ld_xt8(s, n0, n1):
                return lambda q: q.dma_start(
                    out=xt8v[s][:, :, n0:n1],
                    in_=xt8_dram[s][:, :, n0:n1])

            def ld_w(t_sb, t_dram, kn, c0, c1):
                # oc-column slice across all kn chunks of a [128, kn*H] tensor
                tv = t_sb.rearrange("p (k m) -> p k m", k=kn)
                dv = t_dram.rearrange("p (k m) -> p k m", k=kn)
                return lambda q: q.dma_start(out=tv[:, :, c0:c1],
                                             in_=dv[:, :, c0:c1])

            def ld_plain(t_sb, t_dram):
                return lambda q: q.dma_start(out=t_sb[:, :], in_=t_dram[:, :])

            rankedA = [
                # dns b0/b1 PE-critical, in true slot-deadline order
                ld_plain(wbp["dns"], wbp_dram["dns"]),
                ld_w(wt8["dns"], wt8_dram["dns"], K8, 0, 256),
                ld_xt8("dns", 0, 512),
                ld_w(wt16["dns"], wt16_dram["dns"], KB, 0, 256),
                ld_xt16("dns", 4, 6, 0, 512),
                ld_xt16("dns", 6, 8, 0, 512),
                ld_w(wt8["dns"], wt8_dram["dns"], K8, 256, 512),
                ld_w(wt16["dns"], wt16_dram["dns"], KB, 256, 512),
                ld_xt8("dns", 512, 1024),
                ld_xt16("dns", 4, 6, 512, 1024),
                ld_xt16("dns", 6, 8, 512, 1024),
                ld_w(wt8["dns"], wt8_dram["dns"], K8, 512, 1024),
                ld_w(wt16["dns"], wt16_dram["dns"], KB, 512, 768),
                ld_w(wt16["dns"], wt16_dram["dns"], KB, 768, 1024),
            ]
            rankedB = [
                # b2, then img's full proj set, then dns b3 — served by the
                # two HWDGE queues that free up right after rankedA
                ld_xt8("dns", 1024, 1536),
                ld_xt16("dns", 4, 6, 1024, 1536),
                ld_xt16("dns", 6, 8, 1024, 1536),
                ld_plain(wbp["img"], wbp_dram["img"]),
                ld_w(wt8["img"], wt8_dram["img"], K8, 0, 1024),
                ld_xt8("img", 0, 512),
                ld_w(wt16["img"], wt16_dram["img"], KB, 0, 512),
                ld_w(wt16["img"], wt16_dram["img"], KB, 512, 1024),
                ld_xt16("img", 4, 6, 0, 512),
                ld_xt16("img", 6, 8, 0, 512),
                ld_xt8("img", 512, NI),
                ld_xt16("img", 4, 8, 512, NI),
                ld_xt8("dns", 1536, ND),
                ld_xt16("dns", 4, 6, 1536, ND),
                ld_xt16("dns", 6, 8, 1536, ND),
                ld_plain(ident, ident_dram),
                ld_plain(xn3, xn3_dram),
            ]
            rankedC = [
                # Vector-only stage-2 pieces: soft deadlines (they gate only
                # the output DMAs) — confined to the gpsimd queue so they
                # never steal DMA-engine bandwidth from the PE-critical sets
                ld_xt16("dns", 0, 2, 0, 512),
                ld_xt16("dns", 2, 4, 0, 512),
                ld_xt16("dns", 0, 2, 512, 1024),
                ld_xt16("dns", 2, 4, 512, 1024),
                ld_xt16("dns", 0, 2, 1024, 1536),
                ld_xt16("dns", 2, 4, 1024, 1536),
                ld_xt16("img", 0, 2, 0, NI),
                ld_xt16("img", 2, 4, 0, NI),
            ]
            # queue assignment: scalar carries ONLY its small rankedA
            # share (DGE instructions block on ring space — anything more
            # would stall its tanh stream); sync takes the late hard
            # pieces, gpsimd takes the rest + all soft pieces.
            qA = (nc.sync, nc.scalar, nc.gpsimd)
            for i, piece in enumerate(rankedA):
                piece(qA[i % 3])
            sync_tail = [rankedB[i] for i in
                         (0, 2, 4, 5, 7, 10, 12, 13, 14, 15, 16)]
            gp_tail = [rankedB[i] for i in (1, 3, 6, 8, 9, 11)]
            for piece in sync_tail:
                piece(nc.sync)
            for piece in gp_tail:
                piece(nc.gpsimd)
            for piece in rankedC:
                piece(nc.gpsimd)

            # blocks: (side, row0, row1); img in 2 blocks, dns per batch.
            # dns first (smaller per-block DMA appetite smooths the ramp;
            # img's larger working set streams during dns compute); dns b3
            # last purely by need-order of its stage-2 tail.
            blocks = [("dns", 0, 512), ("dns", 512, 1024),
                      ("dns", 1024, 1536),
                      ("img", 0, 512), ("img", 512, NI),
                      ("dns", 1536, 2048)]

            scrap = {s: wpool.tile([128, 512], f16, name=f"scrap_{s}",
                                   tag="scrap", bufs=2) for s in sides}

            gp_scrap = cpool.tile([128, 512], f16, name="gp_scrap")

            def stage2(s, b, aB_tile, off):
                g0 = b * ({"img": R, "dns": S}[s]) - off
                nr = {"img": R, "dns": S}[s]
                for hc in range(HC):
                    nc.vector.scalar_tensor_tensor(
                        out=scrap[s][:, 0:nr],
                        in0=xt16v[s][:, hc, off + g0:off + g0 + nr],
                        scalar=1.0, in1=aB_tile[:, g0:g0 + nr],
                        op0=Alu.mult, op1=Alu.mult,
                        accum_out=oatt_sb[s][:, b * HC + hc:b * HC + hc + 1])

            sc_tiles = {}

            def emit_group_oc(grp, oc):
                """One output-chunk for a group of blocks that SHARE each
                stationary: an LDWEIGHTS swap right after a DoubleRow matmul
                stalls ~190ns for the 256-row drain, so each stationary
                serves every block in the group back-to-back (same-weight
                matmuls issue at the pure streaming rate)."""
                ctx = {}
                for bi in grp:
                    s, n0, n1 = blocks[bi]
                    if oc == 0:
                        sc_tiles[bi] = scps.tile([128, 512], f32,
                                                 name=f"sc_{bi}", tag="sc")
                    ctx[bi] = (s, n0, n1,
                               ppool.tile([128, 512], f32,
                                          name=f"pj_{bi}_{oc}", tag="pp"))
                seq = [("dr", 0), ("dr", 1), ("bf", 0), ("bf", 1),
                       ("bf", 2), ("bf", 3)]
                for si, (kind, k) in enumerate(seq):
                    for bi in grp:
                        s, n0, n1, ps = ctx[bi]
                        n = n1 - n0
                        if kind == "dr":
                            nc.tensor.matmul(
                                ps[0:128, 0:n],
                                lhsT=wt8v[s][:, 2 * k:2 * k + 2,
                                             oc * 128:(oc + 1) * 128],
                                rhs=xt8v[s][:, 2 * k:2 * k + 2, n0:n1],
                                start=(si == 0), stop=False,
                                perf_mode=PM.DoubleRow)
                        else:
                            nc.tensor.matmul(
                                ps[0:128, 0:n],
                                lhsT=wt16v[s][:, k,
                                              oc * 128:(oc + 1) * 128],
                                rhs=xt16v[s][:, K8 + k, n0:n1],
                                start=False, stop=(si == len(seq) - 1))
                for bi in grp:
                    s, n0, n1, ps = ctx[bi]
                    n = n1 - n0
                    th = wpool.tile([128, 512], f16, name=f"th_{bi}_{oc}",
                                    tag="th", bufs=3)
                    nc.scalar.activation(th[:, 0:n], ps[:, 0:n], Act.Tanh,
                                         scale=1.0 / WSC)
                    nc.tensor.matmul(
                        sc_tiles[bi][:, 0:n],
                        lhsT=wbp[s][:, oc:oc + 1].to_broadcast((128, 128)),
                        rhs=th[:, 0:n],
                        start=(oc == 0), stop=(oc == HC - 1))

            def emit_btail(bi):
                s, n0, n1 = blocks[bi]
                n = n1 - n0
                sc_ps = sc_tiles[bi]
                if s == "dns" and n0 // S == BLOC - 1:
                    # final batch: stage-2 on the PE so no Vector work gates
                    # the kernel end.  Per-128 segment: exp -> PE transpose
                    # of the broadcast-row tile (giving the exp'd score
                    # *column*) -> that column (bcast) is the stationary for
                    # weighted row sums over natural-layout rows.
                    NSEG = S // 128
                    for half in range(2):
                        nc.scalar.activation(
                            aB3[:, half * 256:(half + 1) * 256],
                            sc_ps[:, half * 256:(half + 1) * 256], Act.Exp,
                            accum_out=osum_sb[s][:, BLOC + half:BLOC + half + 1])
                    for seg in range(NSEG):
                        tps = scps.tile([128, 512], f32, name=f"tps_{seg}",
                                        tag="sc")
                        nc.tensor.transpose(
                            tps[:, 0:128], aB3[:, seg * 128:(seg + 1) * 128],
                            ident[:, :])
                        nc.scalar.activation(acs3[:, seg:seg + 1],
                                             tps[:, 0:1], Act.Copy)
                    for o2 in range(2):
                        at_ps = ppool.tile([128, 512], f32, name=f"at3_{o2}",
                                           tag="pp")
                        for seg in range(NSEG):
                            nc.tensor.matmul(
                                at_ps[:, :],
                                lhsT=acs3[:, seg:seg + 1].to_broadcast(
                                    (128, 128)),
                                rhs=xn3v[:, seg, o2 * 512:(o2 + 1) * 512],
                                start=(seg == 0), stop=(seg == NSEG - 1))
                        if o2 == 0:
                            nc.vector.tensor_copy(
                                att3[:, 0:512], at_ps[:, :])
                        else:
                            nc.scalar.activation(
                                att3[:, 512:1024], at_ps[:, :], Act.Copy)
                    nc.sync.dma_start(out=out_b3[:, :], in_=att3[0:1, :])
                    nc.sync.dma_start(
                        out=oatt["dns"].rearrange("p b hc -> p (b hc)"),
                        in_=oatt_sb["dns"][:, :])
                    nc.sync.dma_start(out=osum["dns"][:, :],
                                      in_=osum_sb["dns"][:, :])
                elif s == "dns":
                    b = n0 // S
                    aB = wpool.tile([128, S], f16, name=f"aB_d_{b}",
                                    tag="aB_d", bufs=2)
                    nc.scalar.activation(aB[:, :], sc_ps[:, 0:n], Act.Exp,
                                         accum_out=osum_sb[s][:, b:b + 1])
                    stage2("dns", b, aB, b * S)
                else:
                    nc.scalar.activation(aB_img[:, n0:n1], sc_ps[:, 0:n],
                                         Act.Exp)
                    bs = (0, 1) if n1 == 512 else (2, 3)
                    for b in bs:
                        nc.scalar.activation(
                            scrap["img"][:, 0:R],
                            aB_img[:, b * R:(b + 1) * R], Act.Copy,
                            accum_out=osum_sb["img"][:, b:b + 1])
                        stage2("img", b, aB_img, 0)
                    if n1 == NI:
                        nc.sync.dma_start(
                            out=oatt["img"].rearrange("p b hc -> p (b hc)"),
                            in_=oatt_sb["img"][:, :])
                        nc.sync.dma_start(out=osum["img"][:, :],
                                          in_=osum_sb["img"][:, :])

            # interleave the first two dns blocks oc-by-oc: stretches the
            # weight-slice DMA demand over ~2x the time so the ramp stays
            # under the DMA bandwidth ceiling
            # d0/d1/d2 slot-interleaved (stretches the weight-slice DMA
            # demand across ~3x the time -> ramp stays under the DMA
            # bandwidth ceiling); img blk0+blk1 paired (DMA is idle by
            # then, and pairing shares each stationary across both blocks
            # to skip the post-DoubleRow weight-swap stall); d3 solo last
            # (its PE tail ends the kernel).
            order = [(0, 0), (0, 1), (0, 2), (0, 3), (1, 0), (0, 4),
                     (1, 1), (0, 5), (1, 2), (0, 6), (1, 3), (0, 7),
                     "t0", (1, 4), (2, 0), (1, 5), (2, 1), (1, 6),
                     (2, 2), (1, 7), "t1", (2, 3), (2, 4), (2, 5),
                     (2, 6), (2, 7), "t2"]
            for it in order:
                if isinstance(it, str):
                    emit_btail(int(it[1]))
                else:
                    emit_group_oc([it[0]], it[1])
            for oc in range(HC):
                emit_group_oc([3, 4], oc)
            emit_btail(3)
            emit_btail(4)
            for oc in range(HC):
                emit_group_oc([5], oc)
            emit_btail(5)
    nc.compile()
    return nc


def _get_nc():
    if "nc" not in _CACHE:
        _CACHE["nc"] = build_nc()
    return _CACHE["nc"]


def make_in_maps(inputs):
    dns = np.ascontiguousarray(np.asarray(inputs["dns_feature"], np.float32))
    img = np.ascontiguousarray(np.asarray(inputs["img_features"], np.float32))
    W = {"img": np.asarray(inputs["W_i1"], np.float32),
         "dns": np.asarray(inputs["W_d2"], np.float32)}
    wb = {"img": np.asarray(inputs["w_att1"], np.float32)[H:],
          "dns": np.asarray(inputs["w_att2"], np.float32)[H:]}

    wt8 = {}
    wt16 = {}
    wbp = {}
    for s in ("img", "dns"):
        # wt[hc, p, o] = WSC * W[o, hc*128+p]  ->  [128, k*H] host layout
        wt = np.ascontiguousarray(W[s].T * WSC).reshape(HC, 128, H)
        wt8[s] = np.ascontiguousarray(
            wt[:K8].transpose(1, 0, 2).reshape(128, K8 * H)).astype(_E4)
        wt16[s] = np.ascontiguousarray(
            wt[K8:].transpose(1, 0, 2).reshape(128, KB * H)).astype(_BF16)
        wbp[s] = np.ascontiguousarray(wb[s].reshape(HC, 128).T).astype(_BF16)

    in_maps = []
    for k in range(NCORES):
        sl = slice(k * BLOC, (k + 1) * BLOC)
        # [p, k, m] host layouts
        xt_d = dns[sl].reshape(ND, H).T.reshape(HC, 128, ND).transpose(1, 0, 2)
        xt_i = img[sl].reshape(NI, H).T.reshape(HC, 128, NI).transpose(1, 0, 2)
        xn3 = np.ascontiguousarray(
            dns[sl][BLOC - 1].reshape(S // 128, 128, H)
            .transpose(1, 0, 2).reshape(128, (S // 128) * H)).astype(_BF16)
        in_maps.append({
            "ident": np.eye(128, dtype=np.float32),
            "xn3": xn3,
            "xt8_dns": np.ascontiguousarray(xt_d[:, :K8]).astype(_E4),
            "xt8_img": np.ascontiguousarray(xt_i[:, :K8]).astype(_E4),
            "xt16_dns": np.ascontiguousarray(xt_d).astype(_BF16),
            "xt16_img": np.ascontiguousarray(xt_i).astype(_BF16),
            "wt8_img": wt8["img"], "wt8_dns": wt8["dns"],
            "wt16_img": wt16["img"], "wt16_dns": wt16["dns"],
            "wbp_img": wbp["img"], "wbp_dns": wbp["dns"],
        })
    return in_maps


def kernel(**inputs):
    from concourse.bass_utils import run_bass_kernel_spmd

    nc = _get_nc()
    in_maps = make_in_maps(inputs)
    res = run_bass_kernel_spmd(nc, in_maps, list(range(NCORES))).results
    out = {}
    for s in ("img", "dns"):
        per = []
        for k in range(NCORES):
            a = res[k][f"oatt_{s}"].transpose(1, 2, 0).reshape(BLOC, H)
            ssum = res[k][f"osum_{s}"][0].copy()  # same on every partition
            a = a / ssum[:BLOC, None]
            if s == "dns":
                a[BLOC - 1] = res[k]["out_b3"][0] / ssum[BLOC:BLOC + 2].sum()
            per.append(a)
        out[s] = np.concatenate(per, axis=0)
    out_dns = np.ascontiguousarray(
        np.broadcast_to(out["dns"][:, None, :], (B, S, H)).astype(np.float32))
    out_img = np.ascontiguousarray(
        np.broadcast_to(out["img"][:, None, :], (B, S, H)).astype(np.float32))
    return out_dns, out_img
